# revision 68
# baseline (speedup 1.0000x reference)
"""Trainium2 Bass kernel for nn_EquiformerWEdgesBackbone.

Strategy (8 NeuronCores, SPMD, one compiled program):
  - Nodes are degree-balanced into 48 windows of 128 slots (6 windows
    per core) so every window has nearly equal incoming-edge count; a
    core owns the edges whose dst lands in its windows.  Per-dst
    softmax and the segment-sum scatter are fully core-local; the only
    collective is an AllGather of finished gather-table rows once per
    layer.
  - Node state kept TRANSPOSED in SBUF: x_T [C=128 part, (k, n)].
  - Per layer each core builds table rows ONLY for its own 768 slots:
        row = [ y = xn@Wv  (k-major (k,h,v), 1152 cols) | xn0 (128) ]
    and the AllGather concatenates rows into the full table agout
    [6144, 1280] bf16, from which edge tiles gather.
  - Activation-table discipline: the ACT engine only ever loads the
    Exp / Sigmoid / Rsqrt tables (a few per layer); silu(x) is
    decomposed as x*sigmoid(x) with the multiply on DVE, and all
    PSUM->SBUF moves use Identity/Square which live in EVERY table.
  - One-hot [edge,node] tiles are built on-chip (is_equal against an
    iota row) and transposed on PE; no one-hot DMA traffic.
  - Softmax without max-subtraction (logits are O(0.1)); the division
    by the per-(node,head) exp-sum happens once per window after the
    PSUM scatter accumulation.

kernel(**inputs) takes FULL inputs, preprocesses indices on host
(permutation/sort/pad -- no model math), compiles once, runs on cores
0-7, and reassembles the full [N, K, C] float32 output.
"""

import os
import sys
import numpy as np
import ml_dtypes

sys.path.insert(0, "/opt/trn_rl_repo")

import concourse.bass as bass
import concourse.mybir as mybir
import concourse.tile as tile
from concourse import bacc

F32 = mybir.dt.float32
BF16 = mybir.dt.bfloat16
F8 = mybir.dt.float8e4
I16 = mybir.dt.int16
AF = mybir.ActivationFunctionType
MUL = mybir.AluOpType.mult
ADD = mybir.AluOpType.add
ISEQ = mybir.AluOpType.is_equal

# ---------------- problem constants (hardcoded) ----------------
N = 6000
E = 150000
K = 9
C = 128
H = 8
V = 16
L = 2
B = 512
RMAX = 5.0
EPS = 1e-6
GW = RMAX / B
NCORES = 8
NWIN = 6                 # windows per core
NLOCP = NWIN * 128       # 768 slots per core
NSLOT = NCORES * NLOCP   # 6144 global slots
KNP = K * NLOCP          # 6912
YW = K * C               # 1152 y cols, (k, h, v) order
YWP = 1280               # fp8 y row padded to a 256B multiple
TROW = YW + C            # 1280 table row


def _cdiv(a, b):
    return (a + b - 1) // b


def _bc4(t_ap, off_elems):
    """4D zero-stride broadcast AP over (k, h, v) of a [128, >=8] tile:
    in1[p, k, h, v] = t[p, off + h]."""
    return bass.AP(tensor=t_ap.tensor, offset=t_ap.offset + off_elems,
                   ap=[t_ap.ap[0], [0, K], [1, H], [0, V]])


def _gchunks(TW):
    """Per-HALF-window gather chunks (c0, cw): HW dma_gather tops out at
    512 idxs per call; pack 512-chunks then the remainder."""
    GIDX = (TW // 2) * 128
    out = []
    c0 = 0
    while c0 < GIDX:
        cw = min(512, GIDX - c0)
        out.append((c0, cw))
        c0 += cw
    return out


# ============================================================
# program builder
# ============================================================

def build_program(TW, mock_cc=False, reps=1):
    """TW = tiles (128 edge slots) per 128-node window, uniform across
    cores and windows, EVEN.  mock_cc replaces the AllGather with local
    DMAs (single-core cost-model profiling only)."""
    assert TW % 2 == 0
    NT = NWIN * TW           # edge tiles per core
    ECAP = NT * 128          # edge slots per core
    TWE = TW * 128           # edge slots per window
    TW8 = TW * 8
    GCH = _gchunks(TW)       # per-half-window gather chunks
    WCOL = 2 * sum(cw // 16 for _, cw in GCH)   # idx cols per window

    nc = bacc.Bacc("TRN2", target_bir_lowering=False, debug=False,
                   num_devices=NCORES)

    # ---------------- DRAM I/O ----------------
    def din(name, shape, dt):
        return nc.dram_tensor(name, shape, dt, kind="ExternalInput")

    # weights / constants (identical on all cores)
    wrbf1_d = din("wrbf1", [512, C], BF16)
    w2_d = din("wrbf2", [C, C], BF16)
    b1_d = din("brbf1", [C, 1], F32)
    b2_d = din("brbf2", [C, 1], F32)
    cneg_d = din("cneg", [C, 4], F32)          # -centers/GW per basis tile
    atomtab_d = din("atomtab", [40, C], F32)
    bondtab_d = din("bondtab", [24, C], BF16)
    wa1_d = din("wa1", [L, 3, C, C], BF16)     # [l, {a,b,c}, c, c']
    wa2_d = din("wa2", [L, C, H], BF16)
    wg_d = din("wg", [L, C, H], BF16)
    wv_d = din("wv", [L, C, C], BF16)          # columns (h, v) natural
    wo_d = din("wo", [L, C, C], BF16)          # rows (h, v) natural
    wf1_d = din("wf1", [L, C, 512], BF16)
    wf2_d = din("wf2", [L, 512, C], BF16)
    iotac_d = din("iotac", [128, 128], F32)    # row p = [0..127] replicated
    iota8_d = din("iota8", [24, 1], F32)       # p -> p % 8

    # per-core data
    aoh_d = din("aoh", [40, NLOCP], F32)       # atom one-hot (cols = slots)
    d_d = din("dist", [1, ECAP], F32)          # sorted/padded edge distances
    bfr_d = din("bfr", [3, ECAP], BF16)        # bond feature values (pad -1)
    dlnc_d = din("dlnc", [128, NT], F32)       # dst-in-window per tile (pad -1)
    idx_d = din("idx", [128, ECAP // 16], I16)  # wrapped gather indices

    out_d = nc.dram_tensor("xout", [C, KNP], F32, kind="ExternalOutput")

    # internal DRAM: split table: bf16 y rows + bf16 xn0 rows
    aginY = nc.dram_tensor("aginY", [NLOCP, YW], BF16, kind="Internal")
    agoutY = nc.dram_tensor("agoutY", [NSLOT, YW], BF16,
                            kind="Internal", addr_space="Shared")
    aginX = nc.dram_tensor("aginX", [NLOCP, C], BF16, kind="Internal")
    agoutX = nc.dram_tensor("agoutX", [NSLOT, C], BF16,
                            kind="Internal", addr_space="Shared")

    with tile.TileContext(nc) as tc:
        import contextlib
        ctx = contextlib.ExitStack()
        with ctx:
            wp = ctx.enter_context(tc.tile_pool(name="wp", bufs=1))
            big = ctx.enter_context(tc.tile_pool(name="big", bufs=1))

            # ---------- load weights to SBUF ----------
            def wtile(name, shape, dt, src_ap):
                t = wp.tile(shape, dt, tag=name)
                nc.sync.dma_start(out=t[:], in_=src_ap)
                return t

            w1t = wtile("w1t", [128, 4, C], BF16,
                        wrbf1_d.ap().rearrange("(t b) c -> b t c", b=128))
            w2t = wtile("w2t", [C, C], BF16, w2_d.ap())
            b1t = wtile("b1t", [C, 1], F32, b1_d.ap())
            b2t = wtile("b2t", [C, 1], F32, b2_d.ap())
            cnegt = wtile("cnegt", [C, 4], F32, cneg_d.ap())
            atomt = wtile("atomt", [40, C], F32, atomtab_d.ap())
            bondt = wtile("bondt", [24, C], BF16, bondtab_d.ap())
            wa1t = wtile("wa1t", [C, L, 3, C], BF16,
                         wa1_d.ap().rearrange("l t c d -> c l t d"))
            wa2t = wtile("wa2t", [C, L, H], BF16,
                         wa2_d.ap().rearrange("l c h -> c l h"))
            wgt = wtile("wgt", [C, L, H], BF16,
                        wg_d.ap().rearrange("l c h -> c l h"))
            wvt = wtile("wvt", [C, L, C], BF16,
                        wv_d.ap().rearrange("l c d -> c l d"))
            wot = wtile("wot", [C, L, C], BF16,
                        wo_d.ap().rearrange("l c d -> c l d"))
            wf1t = wtile("wf1t", [C, L, 512], BF16,
                         wf1_d.ap().rearrange("l c f -> c l f"))
            wf2t = wtile("wf2t", [128, L, 4, C], BF16,
                         wf2_d.ap().rearrange("l (t b) c -> b l t c", b=128))
            iotart = wtile("iotart", [128, 128], F32, iotac_d.ap())
            iota8t = wtile("iota8t", [24, 1], F32, iota8_d.ap())

            identt = wp.tile([128, 128], BF16, tag="identt")
            from concourse.masks import make_identity
            make_identity(nc, identt[:])
            identf8 = wp.tile([128, 128], F8, tag="identf8")
            nc.scalar.activation(out=identf8[:], in_=identt[:],
                                 func=AF.Identity)
            ones1f = wp.tile([1, 128], F32, tag="ones1f")
            nc.vector.memset(ones1f[:], 1.0)
            ones128b = wp.tile([128, 1], BF16, tag="ones128b")
            nc.vector.memset(ones128b[:], 1.0)
            epst = wp.tile([1, 1], F32, tag="epst")
            nc.vector.memset(epst[:], float(EPS))

            # persistent SBUF state
            x_T = big.tile([C, KNP], F32, tag="x_T")
            e_T = big.tile([C, ECAP], BF16, tag="e_T")
            xn_bf = big.tile([C, KNP], BF16, tag="xn_bf")
            idxs = big.tile([128, ECAP // 16], I16, tag="idxs")
            nc.sync.dma_start(out=idxs[:], in_=idx_d.ap())
            dlncs = big.tile([128, NT], F32, tag="dlncs")
            nc.sync.dma_start(out=dlncs[:], in_=dlnc_d.ap())
            t1w = big.tile([128, NWIN, C], BF16, tag="t1w")
            sig0 = big.tile([128, 4, NLOCP], BF16, tag="sig0")
            ssk = big.tile([1, NLOCP], F32, tag="ssk")
            rs = big.tile([1, NLOCP], F32, tag="rs")

            # ---------- helper: rms_sh normalize x_T -> dst ----------
            def rms_norm(dst_tile):
                with tc.tile_pool(name="rmps", bufs=2, space="PSUM") as rmps, \
                     tc.tile_pool(name="rmsb", bufs=1) as rmsb:
                    scr = rmsb.tile([C, KNP], BF16, tag="scr")
                    # Square is present in every ACT table: no table load.
                    nc.scalar.activation(out=scr[:], in_=x_T[:],
                                         func=AF.Square)
                    # sum over c (partitions, ones-matmul) AND k (PSUM
                    # accumulation).  Keep each output inside one PSUM
                    # bank: split 768 as 512+256.
                    ps = rmps.tile([1, 1024], F32, tag="ss")
                    for k in range(K):
                        for j0, j1 in ((0, 512), (512, NLOCP)):
                            nc.tensor.matmul(
                                ps[:, j0:j1], lhsT=ones128b[:],
                                rhs=scr[:, k * NLOCP + j0:k * NLOCP + j1],
                                start=(k == 0), stop=(k == K - 1))
                    nc.vector.tensor_copy(out=ssk[:], in_=ps[:, 0:NLOCP])
                    nc.scalar.activation(out=rs[:], in_=ssk[:], func=AF.Sqrt,
                                         scale=1.0 / (K * C),
                                         bias=epst[:, 0:1])
                    nc.vector.reciprocal(out=rs[:], in_=rs[:])
                    rb = rmps.tile([C, 1024], F32, tag="rb")
                    nc.tensor.matmul(rb[:, 0:512], lhsT=ones1f[:],
                                     rhs=rs[:, 0:512], start=True, stop=True)
                    nc.tensor.matmul(rb[:, 512:NLOCP], lhsT=ones1f[:],
                                     rhs=rs[:, 512:NLOCP],
                                     start=True, stop=True)
                    nc.vector.tensor_tensor(
                        out=dst_tile[:].rearrange("c (k n) -> c k n", k=K),
                        in0=x_T[:].rearrange("c (k n) -> c k n", k=K),
                        in1=bass.AP(tensor=rb[:].tensor, offset=rb[:].offset,
                                    ap=[rb[:].ap[0], [0, K], [1, NLOCP]]),
                        op=MUL)

            for _rep in range(reps):
                # ---------- P0: init x_T (atom embeddings into l=0) ----------
                with tc.tile_pool(name="p0ps", bufs=2, space="PSUM") as p0ps, \
                     tc.tile_pool(name="p0sb", bufs=1) as p0sb:
                    nc.vector.memset(x_T[:], 0.0)
                    aohs = p0sb.tile([40, NLOCP], F32, tag="aohs")
                    nc.sync.dma_start(out=aohs[:], in_=aoh_d.ap())
                    for j in range(2):
                        ps = p0ps.tile([C, 384], F32, tag="a0")
                        nc.tensor.matmul(ps[:], lhsT=atomt[:],
                                         rhs=aohs[:, j * 384:(j + 1) * 384],
                                         start=True, stop=True)
                        nc.scalar.activation(
                            out=x_T[:, j * 384:(j + 1) * 384], in_=ps[:],
                            func=AF.Identity)

                # ---------- P1: edge features  e_T [c, ECAP] ----------
                # (defined here, invoked after layer_head(0) so it overlaps
                # with the layer-0 collective)
                def run_p1():
                 NCH = ECAP // 512
                 with tc.tile_pool(name="p1ps", bufs=2, space="PSUM") as p1ps, \
                     tc.tile_pool(name="p1qs", bufs=1, space="PSUM") as p1qs, \
                     tc.tile_pool(name="p1st", bufs=1) as p1st, \
                     tc.tile_pool(name="p1sb", bufs=2) as p1sb:
                    h1a = p1st.tile([C, ECAP], BF16, tag="h1a")
                    sga = p1st.tile([C, ECAP], BF16, tag="sga")
                    # pass 1: h1 = W1 @ rbf(dist) + b1  (ACT: Square+Exp only)
                    for ch in range(NCH):
                        sl = slice(ch * 512, (ch + 1) * 512)
                        dbc = p1sb.tile([128, 512], F32, tag="dbc")
                        # SP queue: the Pool queue is busy with the layer-0
                        # collective while P1 runs
                        nc.sync.dma_start(
                            out=dbc[:],
                            in_=bass.AP(tensor=d_d, offset=ch * 512,
                                        ap=[[0, 128], [1, 512]]))
                        sq4 = p1qs.tile([128, 2, 512], F32, tag="sq4")
                        for bt in range(2):
                            nc.scalar.activation(out=sq4[:, bt, :], in_=dbc[:],
                                                 func=AF.Square,
                                                 bias=cnegt[:, bt:bt + 1],
                                                 scale=1.0 / GW)
                        # bt 2/3 squared on DVE (bf16) to unload ACT (the P1
                        # bottleneck): z = d/GW + cneg; sq = z*z
                        sqb = p1sb.tile([128, 2, 512], BF16, tag="sqb")
                        for bt in (2, 3):
                            zt = p1sb.tile([128, 512], BF16, tag="zt")
                            nc.vector.tensor_scalar(
                                out=zt[:], in0=dbc[:], scalar1=1.0 / GW,
                                scalar2=cnegt[:, bt:bt + 1], op0=MUL, op1=ADD)
                            nc.vector.tensor_tensor(out=sqb[:, bt - 2, :],
                                                    in0=zt[:],
                                                    in1=zt[:], op=MUL)
                        rbf4 = p1sb.tile([128, 4, 512], BF16, tag="rbf4")
                        nc.scalar.activation(out=rbf4[:, 0:2, :], in_=sq4[:],
                                             func=AF.Exp, scale=-1.0)
                        nc.scalar.activation(out=rbf4[:, 2:4, :], in_=sqb[:],
                                             func=AF.Exp, scale=-1.0)
                        h1 = p1ps.tile([C, 512], F32, tag="h1")
                        for bt in range(4):
                            nc.tensor.matmul(h1[:], lhsT=w1t[:, bt, :],
                                             rhs=rbf4[:, bt, :],
                                             start=(bt == 0), stop=(bt == 3))
                        nc.vector.tensor_scalar_add(out=h1a[:, sl], in0=h1[:],
                                                    scalar1=b1t[:, 0:1])
                    # silu(h1a) = h1a * sigmoid(h1a): ONE table load
                    nc.scalar.activation(out=sga[:], in_=h1a[:],
                                         func=AF.Sigmoid)
                    nc.vector.tensor_tensor(out=h1a[:], in0=h1a[:],
                                            in1=sga[:], op=MUL)
                    # pass 2: e = hs @ W2 + bond + b2
                    for ch in range(NCH):
                        sl = slice(ch * 512, (ch + 1) * 512)
                        ep = p1ps.tile([C, 512], F32, tag="ep")
                        nc.tensor.matmul(ep[:], lhsT=w2t[:], rhs=h1a[:, sl],
                                         start=True, stop=False)
                        bfc = p1sb.tile([24, 512], BF16, tag="bfc")
                        for f in range(3):
                            nc.sync.dma_start(
                                out=bfc[f * 8:(f + 1) * 8, :],
                                in_=bass.AP(tensor=bfr_d,
                                            offset=f * ECAP + ch * 512,
                                            ap=[[0, 8], [1, 512]]))
                        boh = p1sb.tile([24, 512], BF16, tag="boh")
                        nc.vector.tensor_scalar(out=boh[:], in0=bfc[:],
                                                scalar1=iota8t[:, 0:1],
                                                scalar2=None, op0=ISEQ)
                        nc.tensor.matmul(ep[:], lhsT=bondt[:], rhs=boh[:],
                                         start=False, stop=True)
                        nc.vector.tensor_scalar_add(out=e_T[:, sl], in0=ep[:],
                                                    scalar1=b2t[:, 0:1])

                # ---- layer head: rms + table rows + t1w + AllGather.
                # Called BEFORE P1 for layer 0 so the collective (which
                # occupies the Pool queue) overlaps with the P1 edge MLP.
                def layer_head(l):
                    rms_norm(xn_bf)
                    # table rows for OWN slots: y (fp8, k-major) + xn0 (bf16)
                    with tc.tile_pool(name="tbps", bufs=2,
                                      space="PSUM") as tbps, \
                         tc.tile_pool(name="tbsb", bufs=2) as tbsb:
                        for j in range(NWIN):
                            j0 = j * 128
                            ysb = tbsb.tile([128, YW], BF16, tag="ysb")
                            for k in range(K):
                                yp = tbps.tile([128, C], F32, tag="yp")
                                nc.tensor.matmul(
                                    yp[:],
                                    lhsT=xn_bf[:, k * NLOCP + j0:
                                               k * NLOCP + j0 + 128],
                                    rhs=wvt[:, l, :], start=True, stop=True)
                                nc.scalar.activation(
                                    out=ysb[:, k * C:(k + 1) * C], in_=yp[:],
                                    func=AF.Identity)
                            nc.sync.dma_start(
                                out=aginY.ap()[j0:j0 + 128, :],
                                in_=ysb[:])
                            tp = tbps.tile([128, C], BF16, tag="tp")
                            nc.tensor.transpose(out=tp[:],
                                                in_=xn_bf[:, j0:j0 + 128],
                                                identity=identt[:])
                            xsb = tbsb.tile([128, C], BF16, tag="xsb")
                            nc.scalar.activation(out=xsb[:], in_=tp[:],
                                                 func=AF.Identity)
                            nc.sync.dma_start(out=aginX.ap()[j0:j0 + 128, :],
                                              in_=xsb[:])
                    # t1w per window: [n, c'] = xn0_win.T @ Wa1b
                    with tc.tile_pool(name="t1ps", bufs=2,
                                      space="PSUM") as t1ps:
                        for w in range(NWIN):
                            ps = t1ps.tile([128, C], F32, tag="t1")
                            nc.tensor.matmul(
                                ps[:], lhsT=xn_bf[:, w * 128:(w + 1) * 128],
                                rhs=wa1t[:, l, 1, :], start=True, stop=True)
                            nc.scalar.activation(out=t1w[:, w, :], in_=ps[:],
                                                 func=AF.Identity)
                    if mock_cc == "fast":
                        # perf-sim only: dependency token, no data
                        nc.sync.dma_start(
                            out=bass.AP(tensor=agoutY, offset=0,
                                        ap=[[YW, 128], [1, YW]]),
                            in_=bass.AP(tensor=aginY, offset=0,
                                        ap=[[YW, 128], [1, YW]]))
                        nc.sync.dma_start(
                            out=bass.AP(tensor=agoutX, offset=0,
                                        ap=[[C, 128], [1, C]]),
                            in_=bass.AP(tensor=aginX, offset=0,
                                        ap=[[C, 128], [1, C]]))
                    elif mock_cc:
                        # coarse flat APs: model a byte-bound collective,
                        # not a per-row-descriptor one
                        for dd in range(NCORES):
                            nc.sync.dma_start(
                                out=bass.AP(tensor=agoutY,
                                            offset=dd * NLOCP * YW,
                                            ap=[[YW * 128, NWIN],
                                                [1, YW * 128]]),
                                in_=bass.AP(tensor=aginY, offset=0,
                                            ap=[[YW * 128, NWIN],
                                                [1, YW * 128]]))
                            nc.sync.dma_start(
                                out=bass.AP(tensor=agoutX,
                                            offset=dd * NLOCP * C,
                                            ap=[[C * 128, NWIN],
                                                [1, C * 128]]),
                                in_=bass.AP(tensor=aginX, offset=0,
                                            ap=[[C * 128, NWIN],
                                                [1, C * 128]]))
                    else:
                        nc.gpsimd.collective_compute(
                            "AllGather", mybir.AluOpType.bypass,
                            replica_groups=[list(range(NCORES))],
                            ins=[aginY.ap()], outs=[agoutY.ap()])
                        nc.gpsimd.collective_compute(
                            "AllGather", mybir.AluOpType.bypass,
                            replica_groups=[list(range(NCORES))],
                            ins=[aginX.ap()], outs=[agoutX.ap()])

                layer_head(0)
                run_p1()

                # ---------- per layer ----------
                for l in range(L):
                    if l > 0:
                        layer_head(l)

                    # ---------- edge loop ----------
                    with tc.tile_pool(name="gsb", bufs=1) as gsb, \
                         tc.tile_pool(name="wsb", bufs=1) as wsb, \
                         tc.tile_pool(name="pps", bufs=1,
                                      space="PSUM") as pps, \
                         tc.tile_pool(name="lgps", bufs=1,
                                      space="PSUM") as lgps, \
                         tc.tile_pool(name="agps", bufs=1,
                                      space="PSUM") as agps, \
                         tc.tile_pool(name="tpps", bufs=1,
                                      space="PSUM") as tpps, \
                         tc.tile_pool(name="esb", bufs=3) as esb, \
                         tc.tile_pool(name="episb", bufs=1) as episb:
                        HT = TW // 2
                        GIDX = HT * 128
                        for w in range(NWIN):
                            base_t = w * TW
                            ecol0 = w * TWE
                            ohenW = wsb.tile([128, TWE], BF16, tag="ohenW")
                            ohneW = wsb.tile([128, TWE], BF16, tag="ohneW")
                            preW = wsb.tile([C, TWE], BF16, tag="preW")
                            lg = lgps.tile([128, 512], F32, tag="lg")
                            # gate logits first: only needs e_T, runs on PE
                            # while the gathers below are in flight
                            for ti in range(TW):
                                t = base_t + ti
                                nc.tensor.matmul(
                                    lg[:, ti * 8:(ti + 1) * 8],
                                    lhsT=e_T[:, t * 128:(t + 1) * 128],
                                    rhs=wgt[:, l, :], start=True, stop=True)
                            gtw = episb.tile([128, TW8], BF16, tag="gtw")
                            nc.scalar.activation(out=gtw[:], in_=lg[:, 0:TW8],
                                                 func=AF.Sigmoid)
                            # one-hots for the whole window up front: Pool
                            # work that needs no gather/collective data
                            for ti in range(TW):
                                t = base_t + ti
                                tsl = slice(ti * 128, (ti + 1) * 128)
                                nc.gpsimd.tensor_scalar(
                                    out=ohenW[:, tsl], in0=iotart[:],
                                    scalar1=dlncs[:, t:t + 1],
                                    scalar2=None, op0=ISEQ)
                                tpo = tpps.tile([128, 128], BF16,
                                                tag="tp")
                                nc.tensor.transpose(out=tpo[:],
                                                    in_=ohenW[:, tsl],
                                                    identity=identt[:])
                                nc.scalar.activation(out=ohneW[:, tsl],
                                                     in_=tpo[:],
                                                     func=AF.Identity)
                            # whole-window gather buffers
                            ybufW = gsb.tile([128, TW, YW], BF16,
                                             tag="ybufW")
                            x0bW = gsb.tile([128, TWE], BF16, tag="x0bW")
                            for h in (0, 1):
                                hbase = h * GIDX
                                goff = w * WCOL + h * (GIDX // 16)
                                for c0, cw in GCH:
                                    gsl = slice(goff, goff + cw // 16)
                                    wc = hbase + c0
                                    nc.gpsimd.dma_gather(
                                        ybufW[:, wc // 128:
                                              wc // 128 + cw // 128, :],
                                        agoutY.ap(),
                                        idxs[:, gsl],
                                        cw, cw, YW, elem_step=YW)
                                    nc.gpsimd.dma_gather(
                                        bass.AP(tensor=x0bW[:].tensor,
                                                offset=x0bW[:].offset + wc,
                                                ap=[x0bW[:].ap[0], [cw, 1],
                                                    [1, cw]]),
                                        agoutX.ap(),
                                        idxs[:, gsl],
                                        cw, cw, C, elem_step=C,
                                        transpose=True)
                                    goff += cw // 16
                                # pre chunks + silu (sigmoid reads PSUM)
                                for c0, cw in GCH:
                                    pre = pps.tile([C, 512], F32, tag="pre")
                                    nc.tensor.matmul(
                                        pre[:, 0:cw], lhsT=wa1t[:, l, 2, :],
                                        rhs=e_T[:, ecol0 + hbase + c0:
                                                ecol0 + hbase + c0 + cw],
                                        start=True, stop=False)
                                    nc.tensor.matmul(
                                        pre[:, 0:cw], lhsT=wa1t[:, l, 0, :],
                                        rhs=x0bW[:, hbase + c0:
                                                 hbase + c0 + cw],
                                        start=False, stop=False)
                                    nsub = cw // 128
                                    for s in range(nsub):
                                        ti = h * HT + c0 // 128 + s
                                        nc.tensor.matmul(
                                            pre[:, s * 128:(s + 1) * 128],
                                            lhsT=t1w[:, w, :],
                                            rhs=ohneW[:, ti * 128:
                                                      (ti + 1) * 128],
                                            start=False,
                                            stop=(s == nsub - 1))
                                    nc.scalar.activation(
                                        out=preW[:, hbase + c0:
                                                 hbase + c0 + cw],
                                        in_=pre[:, 0:cw], func=AF.Identity)
                                # silu per half: sigmoid + DVE mult (SBUF)
                                for c0, cw in GCH:
                                    sgc = esb.tile([C, 512], BF16, tag="sgc")
                                    nc.scalar.activation(
                                        out=sgc[:, 0:cw],
                                        in_=preW[:, hbase + c0:
                                                 hbase + c0 + cw],
                                        func=AF.Sigmoid)
                                    nc.vector.tensor_tensor(
                                        out=preW[:, hbase + c0:
                                                 hbase + c0 + cw],
                                        in0=preW[:, hbase + c0:
                                                 hbase + c0 + cw],
                                        in1=sgc[:, 0:cw], op=MUL)
                            # attn logits per tile into lg[:, 256:256+TW8]
                            for ti in range(TW):
                                nc.tensor.matmul(
                                    lg[:, 256 + ti * 8:256 + (ti + 1) * 8],
                                    lhsT=preW[:, ti * 128:(ti + 1) * 128],
                                    rhs=wa2t[:, l, :], start=True, stop=True)
                            exw = episb.tile([128, TW8], BF16, tag="exw")
                            nc.scalar.activation(out=exw[:],
                                                 in_=lg[:, 256:256 + TW8],
                                                 func=AF.Exp)
                            wbw = episb.tile([128, TW8], F32, tag="wbw")
                            nc.vector.tensor_tensor(out=wbw[:], in0=exw[:],
                                                    in1=gtw[:], op=MUL)
                            # scatter: agg[n, (k,h,v)] + sden[n, h]
                            aggs = agps.tile([128, YW], F32, tag="aggs")
                            sden = agps.tile([128, 8], F32, tag="sden")
                            for ti in range(TW):
                                tsl = slice(ti * 128, (ti + 1) * 128)
                                msk = esb.tile([128, YW], BF16, tag="msk")
                                yb0 = ybufW[:, ti, :]
                                nc.vector.tensor_tensor(
                                    out=msk[:].rearrange(
                                        "e (k h v) -> e k h v", k=K, h=H),
                                    in0=yb0.rearrange(
                                        "e (k h v) -> e k h v", k=K, h=H),
                                    in1=_bc4(wbw[:], ti * 8), op=MUL)
                                st = (ti == 0)
                                sp = (ti == TW - 1)
                                nc.tensor.matmul(aggs[:, 0:512],
                                                 lhsT=ohenW[:, tsl],
                                                 rhs=msk[:, 0:512],
                                                 start=st, stop=sp)
                                nc.tensor.matmul(aggs[:, 512:1024],
                                                 lhsT=ohenW[:, tsl],
                                                 rhs=msk[:, 512:1024],
                                                 start=st, stop=sp)
                                nc.tensor.matmul(aggs[:, 1024:YW],
                                                 lhsT=ohenW[:, tsl],
                                                 rhs=msk[:, 1024:YW],
                                                 start=st, stop=sp)
                                nc.tensor.matmul(sden[:],
                                                 lhsT=ohenW[:, tsl],
                                                 rhs=exw[:, ti * 8:
                                                         (ti + 1) * 8],
                                                 start=st, stop=sp)
                            # ----- window epilogue -----
                            rcp = episb.tile([128, H], F32, tag="rcp")
                            nc.vector.tensor_scalar_add(
                                out=rcp[:], in0=sden[:],
                                scalar1=1e-9)
                            nc.vector.reciprocal(out=rcp[:], in_=rcp[:])
                            aggn = episb.tile([128, YW], BF16, tag="aggn")
                            nc.vector.tensor_tensor(
                                out=aggn[:].rearrange(
                                    "n (k h v) -> n k h v", k=K, h=H),
                                in0=aggs[:, 0:YW].rearrange(
                                    "n (k h v) -> n k h v", k=K, h=H),
                                in1=_bc4(rcp[:], 0), op=MUL)
                            for k in range(K):
                                tpe = tpps.tile([128, 128], BF16, tag="tp")
                                nc.tensor.transpose(
                                    out=tpe[:],
                                    in_=aggn[:, k * 128:(k + 1) * 128],
                                    identity=identt[:])
                                aT = esb.tile([128, 128], BF16, tag="aT")
                                nc.scalar.activation(out=aT[:], in_=tpe[:],
                                                     func=AF.Identity)
                                dk = tpps.tile([C, 128], F32, tag="dk")
                                nc.tensor.matmul(dk[:], lhsT=wot[:, l, :],
                                                 rhs=aT[:],
                                                 start=True, stop=True)
                                xsl = slice(k * NLOCP + w * 128,
                                            k * NLOCP + (w + 1) * 128)
                                nc.vector.tensor_tensor(
                                    out=x_T[:, xsl], in0=x_T[:, xsl],
                                    in1=dk[:], op=ADD)

                    # ---------- FFN ----------
                    rms_norm(xn_bf)
                    with tc.tile_pool(name="fps", bufs=2,
                                      space="PSUM") as fps, \
                         tc.tile_pool(name="dps", bufs=2,
                                      space="PSUM") as dps, \
                         tc.tile_pool(name="fsb", bufs=3) as fsb:
                        for j in range(18):
                            c0 = j * 384
                            nsl = slice((j % 2) * 384, (j % 2) * 384 + 384)
                            dlt = dps.tile([C, 384], F32, tag="dlt")
                            for fc in range(4):
                                hp = fps.tile([128, 384], F32, tag="hp")
                                nc.tensor.matmul(
                                    hp[:],
                                    lhsT=wf1t[:, l, fc * 128:(fc + 1) * 128],
                                    rhs=xn_bf[:, c0:c0 + 384],
                                    start=True, stop=True)
                                if j < 2:
                                    nc.scalar.activation(
                                        out=sig0[:, fc, nsl],
                                        in_=hp[:], func=AF.Sigmoid)
                                hb = fsb.tile([128, 384], BF16, tag="hb")
                                nc.vector.tensor_tensor(
                                    out=hb[:], in0=hp[:],
                                    in1=sig0[:, fc, nsl], op=MUL)
                                nc.tensor.matmul(
                                    dlt[:], lhsT=wf2t[:, l, fc, :], rhs=hb[:],
                                    start=(fc == 0), stop=(fc == 3))
                            nc.vector.tensor_tensor(
                                out=x_T[:, c0:c0 + 384],
                                in0=x_T[:, c0:c0 + 384],
                                in1=dlt[:], op=ADD)

                # ---------- final norm + output ----------
                rms_norm(x_T)
                nc.sync.dma_start(out=out_d.ap(), in_=x_T[:])

    nc.compile()
    return nc


# ============================================================
# host preprocessing + runner
# ============================================================

_CACHE = {}


def _prep(inputs):
    """Index-only host preprocessing; returns (TW, in_maps, gslot)."""
    atom_feats = np.asarray(inputs["atom_feats"]).astype(np.int64)
    bond_feats = np.asarray(inputs["bond_feats"]).astype(np.int64)
    edge_index = np.asarray(inputs["edge_index"]).astype(np.int64)
    edge_distance = np.asarray(inputs["edge_distance"]).astype(np.float32)

    src, dst = edge_index[0], edge_index[1]
    NW = NCORES * NWIN            # 48 windows global

    # ---- degree-balanced node -> (window, slot) assignment ----
    deg = np.bincount(dst, minlength=N)
    order_nodes = np.argsort(-deg, kind="stable")
    win = np.empty(N, np.int64)
    pos = np.empty(N, np.int64)
    r = np.arange(N)
    win[order_nodes] = r % NW
    pos[order_nodes] = r // NW     # 0..124 (125 nodes per window)
    gslot = win * 128 + pos        # global slot id, 0..6143

    wl = np.bincount(win[dst], minlength=NW)
    TW = _cdiv(int(wl.max()), 128)
    TW += TW % 2                   # even (gather half-windows)
    NT = NWIN * TW
    ECAP = NT * 128
    TWE = TW * 128

    # ---- edge -> slot assignment (sorted by (window, src)) ----
    wid_e = win[dst]
    order_e = np.lexsort((gslot[src], wid_e))
    we_s = wid_e[order_e]
    starts = np.zeros(NW + 1, np.int64)
    np.cumsum(np.bincount(we_s, minlength=NW), out=starts[1:])
    rank = np.arange(E) - starts[we_s]
    eslot_g = we_s * TWE + rank            # global edge-slot (48*TWE space)

    srcslot = np.zeros(NW * TWE, np.int64)
    dlnf = np.full(NW * TWE, -1.0, np.float32)
    distf = np.zeros(NW * TWE, np.float32)
    bff = np.full((3, NW * TWE), -1.0, np.float32)
    srcslot[eslot_g] = gslot[src[order_e]]
    dlnf[eslot_g] = (gslot[dst[order_e]] % 128).astype(np.float32)
    distf[eslot_g] = edge_distance[order_e]
    bff[:, eslot_g] = bond_feats[order_e].T.astype(np.float32)

    # ---- weights ----
    f32 = np.float32
    bf16 = ml_dtypes.bfloat16
    W_rbf1 = np.asarray(inputs["W_rbf1"], f32)
    W_rbf2 = np.asarray(inputs["W_rbf2"], f32)
    b_rbf1 = np.asarray(inputs["b_rbf1"], f32)
    b_rbf2 = np.asarray(inputs["b_rbf2"], f32)
    Wa1 = np.asarray(inputs["Wa1"], f32)
    Wa2 = np.asarray(inputs["Wa2"], f32)
    Wv = np.asarray(inputs["Wv"], f32)
    Wg = np.asarray(inputs["Wg"], f32)
    Wo = np.asarray(inputs["Wo"], f32)
    Wf1 = np.asarray(inputs["Wf1"], f32)
    Wf2 = np.asarray(inputs["Wf2"], f32)
    atom_emb = np.asarray(inputs["atom_emb"], f32)
    bond_emb = np.asarray(inputs["bond_emb"], f32)

    centers = np.linspace(0.0, RMAX, B).astype(f32)
    cneg = (-centers / GW).reshape(4, 128).T.copy()       # [128, 4]

    common = {
        "wrbf1": W_rbf1.astype(bf16),
        "wrbf2": W_rbf2.astype(bf16),
        "brbf1": b_rbf1.reshape(C, 1),
        "brbf2": b_rbf2.reshape(C, 1),
        "cneg": np.ascontiguousarray(cneg),
        "atomtab": atom_emb.reshape(40, C).copy(),
        "bondtab": bond_emb.reshape(24, C).astype(bf16),
        "wa1": np.ascontiguousarray(Wa1.reshape(L, 3, C, C)).astype(bf16),
        "wa2": Wa2.astype(bf16),
        "wg": Wg.astype(bf16),
        "wv": Wv.astype(bf16),
        "wo": Wo.astype(bf16),
        "wf1": Wf1.astype(bf16),
        "wf2": Wf2.astype(bf16),
        "iotac": np.tile(np.arange(128, dtype=f32), (128, 1)),
        "iota8": (np.arange(24) % 8).astype(f32).reshape(24, 1),
    }

    # atom one-hot per core (cols = local slots, pads stay zero)
    core_n = gslot // NLOCP
    loc_n = gslot % NLOCP
    aoh_all = np.zeros((NCORES, 40, NLOCP), f32)
    for f in range(4):
        aoh_all[core_n, f * 10 + atom_feats[:, f], loc_n] = 1.0

    # wrapped int16 gather indices: per gather chunk (<=512 idxs), idxs
    # wrap into 16 rows x cw/16 cols, chunks packed consecutively,
    # replicated to 128 partitions.
    gch = _gchunks(TW)
    in_maps = []
    for d in range(NCORES):
        sl = slice(d * ECAP, (d + 1) * ECAP)
        ss = srcslot[sl].astype(np.int16)
        idx = np.zeros((16, ECAP // 16), np.int16)
        gidx = (TW // 2) * 128
        col = 0
        for w in range(NWIN):
            for h in (0, 1):
                for c0, cw in gch:
                    base = w * TWE + h * gidx + c0
                    q = np.arange(cw)
                    idx[q % 16, col + q // 16] = ss[base:base + cw]
                    col += cw // 16
        idx = np.tile(idx, (8, 1))

        m = dict(common)
        m.update({
            "aoh": aoh_all[d],
            "dist": distf[sl].reshape(1, ECAP),
            "bfr": np.ascontiguousarray(bff[:, sl]).astype(bf16),
            "dlnc": np.ascontiguousarray(
                dlnf[sl].reshape(NT, 128).T),
            "idx": idx,
        })
        in_maps.append(m)
    return TW, in_maps, gslot


def _get_nc(TW, reps=1):
    key = ('nc', TW, reps)
    if key not in _CACHE:
        _CACHE[key] = build_program(TW, reps=reps)
    return _CACHE[key]


def _make_runner(nc, reps):
    """jit-compiled SPMD runner that chains the NEFF `reps` times
    back-to-back, so (T(reps) - T(1)) / (reps - 1) cancels host/axon
    dispatch overhead."""
    import jax
    from jax.sharding import Mesh, PartitionSpec
    from jax.experimental.shard_map import shard_map
    from concourse import bass2jax
    import concourse.mybir as mb

    bass2jax.install_neuronx_cc_hook()
    part_name = (nc.partition_id_tensor.name
                 if nc.partition_id_tensor else None)
    in_names, out_names, out_avals, zero_outs = [], [], [], []
    for alloc in nc.m.functions[0].allocations:
        if not isinstance(alloc, mybir.MemoryLocationSet):
            continue
        name = alloc.memorylocations[0].name
        if alloc.kind == "ExternalInput":
            if name != part_name:
                in_names.append(name)
        elif alloc.kind == "ExternalOutput":
            out_names.append(name)
            shape = tuple(alloc.tensor_shape)
            dtype = mb.dt.np(alloc.dtype)
            out_avals.append(jax.core.ShapedArray(shape, dtype))
            zero_outs.append(np.zeros(shape, dtype))
    n_params = len(in_names)
    all_names = list(in_names) + list(out_names)
    if part_name is not None:
        all_names.append(part_name)

    def _body(*args):
        o = list(args[n_params:])
        for _ in range(reps):
            ops = list(args[:n_params]) + o
            if part_name is not None:
                ops.append(bass2jax.partition_id_tensor())
            o = list(bass2jax._bass_exec_p.bind(
                *ops,
                out_avals=tuple(out_avals),
                in_names=tuple(all_names),
                out_names=tuple(out_names),
                lowering_input_output_aliases=(),
                sim_require_finite=True,
                sim_require_nnan=True,
                nc=nc))
        return tuple(o)

    devices = jax.devices()[:NCORES]
    mesh = Mesh(np.asarray(devices), ("core",))
    n_outs = len(out_names)
    in_specs = (PartitionSpec("core"),) * (n_params + n_outs)
    out_specs = (PartitionSpec("core"),) * n_outs
    donate = tuple(range(n_params, n_params + n_outs))
    fn = jax.jit(
        shard_map(_body, mesh=mesh, in_specs=in_specs,
                  out_specs=out_specs, check_rep=False),
        donate_argnums=donate, keep_unused=True)
    return fn, mesh, in_names, out_names, out_avals, zero_outs


def _concat_inputs(in_maps, in_names):
    return [np.concatenate([np.asarray(in_maps[c][n]) for c in range(NCORES)],
                           axis=0) for n in in_names]


def _unpack_out(arrs, gslot):
    # arrs[0]: [NCORES*C, KNP] -> full [N, K, C]
    xo = np.asarray(arrs[0]).reshape(NCORES, C, K, NLOCP)
    flat = np.ascontiguousarray(xo.transpose(0, 3, 2, 1)).reshape(
        NSLOT, K, C)
    return flat[gslot].astype(np.float32)


def _run(TW, in_maps, gslot, reps_timing=0):
    """Returns (out, timing_info)."""
    import jax, time
    nc = _get_nc(TW)
    key = (TW, 1)
    if key not in _CACHE:
        _CACHE[key] = _make_runner(nc, 1)
    fn1, mesh, in_names, out_names, out_avals, zero_outs = _CACHE[key]
    cin = _concat_inputs(in_maps, in_names)
    czo = [np.zeros((NCORES * z.shape[0], *z.shape[1:]), z.dtype)
           for z in zero_outs]
    outs = fn1(*cin, *czo)
    jax.block_until_ready(outs)
    result = _unpack_out(outs, gslot)

    timing = None
    if reps_timing:
        from jax.sharding import NamedSharding, PartitionSpec
        shard = NamedSharding(mesh, PartitionSpec("core"))
        cin_dev = [jax.device_put(a, shard) for a in cin]

        def seq_times(f, n):
            ts = []
            for _ in range(n):
                z = [jax.device_put(np.zeros_like(a), shard) for a in czo]
                jax.block_until_ready(z)
                t0 = time.perf_counter()
                o = f(*cin_dev, *z)
                jax.block_until_ready(o)
                ts.append(time.perf_counter() - t0)
            return ts

        seq_times(fn1, 2)  # warm
        ts1 = seq_times(fn1, 10)
        R = reps_timing
        tsR = None
        if R > 1:
            ncR = _get_nc(TW, reps=R)
            kr = ("fn", TW, R)
            if kr not in _CACHE:
                _CACHE[kr] = _make_runner(ncR, 1)
            fnR = _CACHE[kr][0]
            seq_times(fnR, 1)  # warm/compile
            tsR = seq_times(fnR, 10)
            med = lambda v: sorted(v)[len(v) // 2]
            per_iter = max(med(tsR) - med(ts1), 0.0) / (R - 1)
        else:
            per_iter = min(ts1)
        timing = dict(ts1=ts1, tsR=tsR, reps=R, per_iter=per_iter)
    return result, timing


def kernel(**inputs):
    TW, in_maps, gslot = _prep(inputs)
    out, _ = _run(TW, in_maps, gslot)
    return out


# revision 69
# speedup vs baseline: 1.0130x; 1.0130x over previous
"""Trainium2 Bass kernel for nn_EquiformerWEdgesBackbone.

Strategy (8 NeuronCores, SPMD, one compiled program):
  - Nodes are degree-balanced into 48 windows of 128 slots (6 windows
    per core) so every window has nearly equal incoming-edge count; a
    core owns the edges whose dst lands in its windows.  Per-dst
    softmax and the segment-sum scatter are fully core-local; the only
    collective is an AllGather of finished gather-table rows once per
    layer.
  - Node state kept TRANSPOSED in SBUF: x_T [C=128 part, (k, n)].
  - Per layer each core builds table rows ONLY for its own 768 slots:
        row = [ y = xn@Wv  (k-major (k,h,v), 1152 cols) | xn0 (128) ]
    and the AllGather concatenates rows into the full table agout
    [6144, 1280] bf16, from which edge tiles gather.
  - Activation-table discipline: the ACT engine only ever loads the
    Exp / Sigmoid / Rsqrt tables (a few per layer); silu(x) is
    decomposed as x*sigmoid(x) with the multiply on DVE, and all
    PSUM->SBUF moves use Identity/Square which live in EVERY table.
  - One-hot [edge,node] tiles are built on-chip (is_equal against an
    iota row) and transposed on PE; no one-hot DMA traffic.
  - Softmax without max-subtraction (logits are O(0.1)); the division
    by the per-(node,head) exp-sum happens once per window after the
    PSUM scatter accumulation.

kernel(**inputs) takes FULL inputs, preprocesses indices on host
(permutation/sort/pad -- no model math), compiles once, runs on cores
0-7, and reassembles the full [N, K, C] float32 output.
"""

import os
import sys
import numpy as np
import ml_dtypes

sys.path.insert(0, "/opt/trn_rl_repo")

import concourse.bass as bass
import concourse.mybir as mybir
import concourse.tile as tile
from concourse import bacc

F32 = mybir.dt.float32
BF16 = mybir.dt.bfloat16
I16 = mybir.dt.int16
AF = mybir.ActivationFunctionType
MUL = mybir.AluOpType.mult
ADD = mybir.AluOpType.add
ISEQ = mybir.AluOpType.is_equal

# ---------------- problem constants (hardcoded) ----------------
N = 6000
E = 150000
K = 9
C = 128
H = 8
V = 16
L = 2
B = 512
RMAX = 5.0
EPS = 1e-6
GW = RMAX / B
NCORES = 8
NWIN = 6                 # windows per core
NLOCP = NWIN * 128       # 768 slots per core
NSLOT = NCORES * NLOCP   # 6144 global slots
KNP = K * NLOCP          # 6912
YW = K * C               # 1152 y cols, (k, h, v) order
TROW = YW + C            # 1280 table row


def _cdiv(a, b):
    return (a + b - 1) // b


def _bc4(t_ap, off_elems):
    """4D zero-stride broadcast AP over (k, h, v) of a [128, >=8] tile:
    in1[p, k, h, v] = t[p, off + h]."""
    return bass.AP(tensor=t_ap.tensor, offset=t_ap.offset + off_elems,
                   ap=[t_ap.ap[0], [0, K], [1, H], [0, V]])


def _gchunks(TW):
    """Per-window gather chunks (c0, cw): HW dma_gather tops out at 512
    idxs per call; pack 512-chunks then the remainder."""
    TWE = TW * 128
    out = []
    c0 = 0
    while c0 < TWE:
        cw = min(512, TWE - c0)
        out.append((c0, cw))
        c0 += cw
    return out


# ============================================================
# program builder
# ============================================================

def build_program(TW, mock_cc=False, reps=1):
    """TW = tiles (128 edge slots) per 128-node window, uniform across
    cores and windows, EVEN.  mock_cc replaces the AllGather with local
    DMAs (single-core cost-model profiling only)."""
    assert TW % 2 == 0
    NT = NWIN * TW           # edge tiles per core
    ECAP = NT * 128          # edge slots per core
    TWE = TW * 128           # edge slots per window
    TW8 = TW * 8
    GCH = _gchunks(TW)       # per-window gather chunks
    WCOL = sum(cw // 16 for _, cw in GCH)   # idx cols per window

    nc = bacc.Bacc("TRN2", target_bir_lowering=False, debug=False,
                   num_devices=NCORES)

    # ---------------- DRAM I/O ----------------
    def din(name, shape, dt):
        return nc.dram_tensor(name, shape, dt, kind="ExternalInput")

    # weights / constants (identical on all cores)
    wrbf1_d = din("wrbf1", [512, C], BF16)
    w2_d = din("wrbf2", [C, C], BF16)
    b1_d = din("brbf1", [C, 1], F32)
    b2_d = din("brbf2", [C, 1], F32)
    cneg_d = din("cneg", [C, 4], F32)          # -centers/GW per basis tile
    atomtab_d = din("atomtab", [40, C], F32)
    bondtab_d = din("bondtab", [24, C], BF16)
    wa1_d = din("wa1", [L, 3, C, C], BF16)     # [l, {a,b,c}, c, c']
    wa2_d = din("wa2", [L, C, H], BF16)
    wg_d = din("wg", [L, C, H], BF16)
    wv_d = din("wv", [L, C, C], BF16)          # columns (h, v) natural
    wo_d = din("wo", [L, C, C], BF16)          # rows (h, v) natural
    wf1_d = din("wf1", [L, C, 512], BF16)
    wf2_d = din("wf2", [L, 512, C], BF16)
    iotac_d = din("iotac", [128, 128], F32)    # row p = [0..127] replicated
    iota8_d = din("iota8", [24, 1], F32)       # p -> p % 8

    # per-core data
    aoh_d = din("aoh", [40, NLOCP], F32)       # atom one-hot (cols = slots)
    d_d = din("dist", [1, ECAP], F32)          # sorted/padded edge distances
    bfr_d = din("bfr", [3, ECAP], F32)         # bond feature values (pad -1)
    dlnc_d = din("dlnc", [128, NT], F32)       # dst-in-window per tile (pad -1)
    idx_d = din("idx", [128, ECAP // 16], I16)  # wrapped gather indices

    out_d = nc.dram_tensor("xout", [C, KNP], F32, kind="ExternalOutput")

    # internal DRAM
    agin = nc.dram_tensor("agin", [NLOCP, TROW], BF16, kind="Internal")
    agout = nc.dram_tensor("agout", [NSLOT, TROW], BF16,
                           kind="Internal", addr_space="Shared")

    with tile.TileContext(nc) as tc:
        import contextlib
        ctx = contextlib.ExitStack()
        with ctx:
            wp = ctx.enter_context(tc.tile_pool(name="wp", bufs=1))
            big = ctx.enter_context(tc.tile_pool(name="big", bufs=1))

            # ---------- load weights to SBUF ----------
            def wtile(name, shape, dt, src_ap):
                t = wp.tile(shape, dt, tag=name)
                nc.sync.dma_start(out=t[:], in_=src_ap)
                return t

            w1t = wtile("w1t", [128, 4, C], BF16,
                        wrbf1_d.ap().rearrange("(t b) c -> b t c", b=128))
            w2t = wtile("w2t", [C, C], BF16, w2_d.ap())
            b1t = wtile("b1t", [C, 1], F32, b1_d.ap())
            b2t = wtile("b2t", [C, 1], F32, b2_d.ap())
            cnegt = wtile("cnegt", [C, 4], F32, cneg_d.ap())
            atomt = wtile("atomt", [40, C], F32, atomtab_d.ap())
            bondt = wtile("bondt", [24, C], BF16, bondtab_d.ap())
            wa1t = wtile("wa1t", [C, L, 3, C], BF16,
                         wa1_d.ap().rearrange("l t c d -> c l t d"))
            wa2t = wtile("wa2t", [C, L, H], BF16,
                         wa2_d.ap().rearrange("l c h -> c l h"))
            wgt = wtile("wgt", [C, L, H], BF16,
                        wg_d.ap().rearrange("l c h -> c l h"))
            wvt = wtile("wvt", [C, L, C], BF16,
                        wv_d.ap().rearrange("l c d -> c l d"))
            wot = wtile("wot", [C, L, C], BF16,
                        wo_d.ap().rearrange("l c d -> c l d"))
            wf1t = wtile("wf1t", [C, L, 512], BF16,
                         wf1_d.ap().rearrange("l c f -> c l f"))
            wf2t = wtile("wf2t", [128, L, 4, C], BF16,
                         wf2_d.ap().rearrange("l (t b) c -> b l t c", b=128))
            iotart = wtile("iotart", [128, 128], F32, iotac_d.ap())
            iota8t = wtile("iota8t", [24, 1], F32, iota8_d.ap())

            identt = wp.tile([128, 128], BF16, tag="identt")
            from concourse.masks import make_identity
            make_identity(nc, identt[:])
            ones1f = wp.tile([1, 128], F32, tag="ones1f")
            nc.vector.memset(ones1f[:], 1.0)
            ones128b = wp.tile([128, 1], BF16, tag="ones128b")
            nc.vector.memset(ones128b[:], 1.0)
            epst = wp.tile([1, 1], F32, tag="epst")
            nc.vector.memset(epst[:], float(EPS))

            # persistent SBUF state
            x_T = big.tile([C, KNP], F32, tag="x_T")
            e_T = big.tile([C, ECAP], BF16, tag="e_T")
            xn_bf = big.tile([C, KNP], BF16, tag="xn_bf")
            idxs = big.tile([128, ECAP // 16], I16, tag="idxs")
            nc.sync.dma_start(out=idxs[:], in_=idx_d.ap())
            dlncs = big.tile([128, NT], F32, tag="dlncs")
            nc.sync.dma_start(out=dlncs[:], in_=dlnc_d.ap())
            t1w = big.tile([128, NWIN, C], BF16, tag="t1w")
            sig0 = big.tile([128, 4, NLOCP], BF16, tag="sig0")
            ssk = big.tile([1, NLOCP], F32, tag="ssk")
            rs = big.tile([1, NLOCP], F32, tag="rs")

            # ---------- helper: rms_sh normalize x_T -> dst ----------
            def rms_norm(dst_tile):
                with tc.tile_pool(name="rmps", bufs=2, space="PSUM") as rmps, \
                     tc.tile_pool(name="rmsb", bufs=1) as rmsb:
                    scr = rmsb.tile([C, KNP], BF16, tag="scr")
                    # Square is present in every ACT table: no table load.
                    nc.scalar.activation(out=scr[:], in_=x_T[:],
                                         func=AF.Square)
                    # sum over c (partitions, ones-matmul) AND k (PSUM
                    # accumulation).  Keep each output inside one PSUM
                    # bank: split 768 as 512+256.
                    ps = rmps.tile([1, 1024], F32, tag="ss")
                    for k in range(K):
                        for j0, j1 in ((0, 512), (512, NLOCP)):
                            nc.tensor.matmul(
                                ps[:, j0:j1], lhsT=ones128b[:],
                                rhs=scr[:, k * NLOCP + j0:k * NLOCP + j1],
                                start=(k == 0), stop=(k == K - 1))
                    nc.vector.tensor_copy(out=ssk[:], in_=ps[:, 0:NLOCP])
                    nc.scalar.activation(out=rs[:], in_=ssk[:], func=AF.Sqrt,
                                         scale=1.0 / (K * C),
                                         bias=epst[:, 0:1])
                    nc.vector.reciprocal(out=rs[:], in_=rs[:])
                    rb = rmps.tile([C, 1024], F32, tag="rb")
                    nc.tensor.matmul(rb[:, 0:512], lhsT=ones1f[:],
                                     rhs=rs[:, 0:512], start=True, stop=True)
                    nc.tensor.matmul(rb[:, 512:NLOCP], lhsT=ones1f[:],
                                     rhs=rs[:, 512:NLOCP],
                                     start=True, stop=True)
                    nc.vector.tensor_tensor(
                        out=dst_tile[:].rearrange("c (k n) -> c k n", k=K),
                        in0=x_T[:].rearrange("c (k n) -> c k n", k=K),
                        in1=bass.AP(tensor=rb[:].tensor, offset=rb[:].offset,
                                    ap=[rb[:].ap[0], [0, K], [1, NLOCP]]),
                        op=MUL)

            for _rep in range(reps):
                # ---------- P0: init x_T (atom embeddings into l=0) ----------
                with tc.tile_pool(name="p0ps", bufs=2, space="PSUM") as p0ps, \
                     tc.tile_pool(name="p0sb", bufs=1) as p0sb:
                    nc.vector.memset(x_T[:], 0.0)
                    aohs = p0sb.tile([40, NLOCP], F32, tag="aohs")
                    nc.sync.dma_start(out=aohs[:], in_=aoh_d.ap())
                    for j in range(2):
                        ps = p0ps.tile([C, 384], F32, tag="a0")
                        nc.tensor.matmul(ps[:], lhsT=atomt[:],
                                         rhs=aohs[:, j * 384:(j + 1) * 384],
                                         start=True, stop=True)
                        nc.scalar.activation(
                            out=x_T[:, j * 384:(j + 1) * 384], in_=ps[:],
                            func=AF.Identity)

                # ---------- P1: edge features  e_T [c, ECAP] ----------
                NCH = ECAP // 512
                with tc.tile_pool(name="p1ps", bufs=2, space="PSUM") as p1ps, \
                     tc.tile_pool(name="p1qs", bufs=1, space="PSUM") as p1qs, \
                     tc.tile_pool(name="p1st", bufs=1) as p1st, \
                     tc.tile_pool(name="p1sb", bufs=2) as p1sb:
                    h1a = p1st.tile([C, ECAP], BF16, tag="h1a")
                    sga = p1st.tile([C, ECAP], BF16, tag="sga")
                    # pass 1: h1 = W1 @ rbf(dist) + b1  (ACT: Square+Exp only)
                    for ch in range(NCH):
                        sl = slice(ch * 512, (ch + 1) * 512)
                        dbc = p1sb.tile([128, 512], F32, tag="dbc")
                        nc.gpsimd.dma_start(
                            out=dbc[:],
                            in_=bass.AP(tensor=d_d, offset=ch * 512,
                                        ap=[[0, 128], [1, 512]]))
                        sq4 = p1qs.tile([128, 4, 512], F32, tag="sq4")
                        for bt in range(4):
                            nc.scalar.activation(out=sq4[:, bt, :], in_=dbc[:],
                                                 func=AF.Square,
                                                 bias=cnegt[:, bt:bt + 1],
                                                 scale=1.0 / GW)
                        rbf4 = p1sb.tile([128, 4, 512], BF16, tag="rbf4")
                        nc.scalar.activation(out=rbf4[:], in_=sq4[:],
                                             func=AF.Exp, scale=-1.0)
                        h1 = p1ps.tile([C, 512], F32, tag="h1")
                        for bt in range(4):
                            nc.tensor.matmul(h1[:], lhsT=w1t[:, bt, :],
                                             rhs=rbf4[:, bt, :],
                                             start=(bt == 0), stop=(bt == 3))
                        nc.vector.tensor_scalar_add(out=h1a[:, sl], in0=h1[:],
                                                    scalar1=b1t[:, 0:1])
                    # silu(h1a) = h1a * sigmoid(h1a): ONE table load
                    nc.scalar.activation(out=sga[:], in_=h1a[:],
                                         func=AF.Sigmoid)
                    nc.vector.tensor_tensor(out=h1a[:], in0=h1a[:],
                                            in1=sga[:], op=MUL)
                    # pass 2: e = hs @ W2 + bond + b2
                    for ch in range(NCH):
                        sl = slice(ch * 512, (ch + 1) * 512)
                        ep = p1ps.tile([C, 512], F32, tag="ep")
                        nc.tensor.matmul(ep[:], lhsT=w2t[:], rhs=h1a[:, sl],
                                         start=True, stop=False)
                        bfc = p1sb.tile([24, 512], F32, tag="bfc")
                        for f in range(3):
                            nc.sync.dma_start(
                                out=bfc[f * 8:(f + 1) * 8, :],
                                in_=bass.AP(tensor=bfr_d,
                                            offset=f * ECAP + ch * 512,
                                            ap=[[0, 8], [1, 512]]))
                        boh = p1sb.tile([24, 512], BF16, tag="boh")
                        nc.vector.tensor_scalar(out=boh[:], in0=bfc[:],
                                                scalar1=iota8t[:, 0:1],
                                                scalar2=None, op0=ISEQ)
                        nc.tensor.matmul(ep[:], lhsT=bondt[:], rhs=boh[:],
                                         start=False, stop=True)
                        nc.vector.tensor_scalar_add(out=e_T[:, sl], in0=ep[:],
                                                    scalar1=b2t[:, 0:1])

                # ---------- per layer ----------
                for l in range(L):
                    rms_norm(xn_bf)

                    # table rows for OWN slots: [y (k-major) | xn0]
                    with tc.tile_pool(name="tbps", bufs=2,
                                      space="PSUM") as tbps, \
                         tc.tile_pool(name="tbsb", bufs=2) as tbsb:
                        for j in range(NWIN):
                            j0 = j * 128
                            ysb = tbsb.tile([128, TROW], BF16, tag="ysb")
                            for k in range(K):
                                yp = tbps.tile([128, C], F32, tag="yp")
                                nc.tensor.matmul(
                                    yp[:],
                                    lhsT=xn_bf[:, k * NLOCP + j0:
                                               k * NLOCP + j0 + 128],
                                    rhs=wvt[:, l, :], start=True, stop=True)
                                nc.scalar.activation(
                                    out=ysb[:, k * C:(k + 1) * C], in_=yp[:],
                                    func=AF.Identity)
                            tp = tbps.tile([128, C], BF16, tag="tp")
                            nc.tensor.transpose(out=tp[:],
                                                in_=xn_bf[:, j0:j0 + 128],
                                                identity=identt[:])
                            nc.scalar.activation(out=ysb[:, YW:TROW],
                                                 in_=tp[:], func=AF.Identity)
                            nc.sync.dma_start(out=agin.ap()[j0:j0 + 128, :],
                                              in_=ysb[:])

                    if mock_cc:
                        for dd in range(NCORES):
                            nc.sync.dma_start(
                                out=agout.ap()[dd * NLOCP:(dd + 1) * NLOCP, :],
                                in_=agin.ap())
                    else:
                        nc.gpsimd.collective_compute(
                            "AllGather", mybir.AluOpType.bypass,
                            replica_groups=[list(range(NCORES))],
                            ins=[agin.ap()], outs=[agout.ap()])

                    # t1w per window: [n, c'] = xn0_win.T @ Wa1b
                    with tc.tile_pool(name="t1ps", bufs=2,
                                      space="PSUM") as t1ps:
                        for w in range(NWIN):
                            ps = t1ps.tile([128, C], F32, tag="t1")
                            nc.tensor.matmul(
                                ps[:], lhsT=xn_bf[:, w * 128:(w + 1) * 128],
                                rhs=wa1t[:, l, 1, :], start=True, stop=True)
                            nc.scalar.activation(out=t1w[:, w, :], in_=ps[:],
                                                 func=AF.Identity)

                    # ---------- edge loop ----------
                    with tc.tile_pool(name="gsb", bufs=1) as gsb, \
                         tc.tile_pool(name="wsb", bufs=1) as wsb, \
                         tc.tile_pool(name="pps", bufs=1,
                                      space="PSUM") as pps, \
                         tc.tile_pool(name="lgps", bufs=1,
                                      space="PSUM") as lgps, \
                         tc.tile_pool(name="agps", bufs=1,
                                      space="PSUM") as agps, \
                         tc.tile_pool(name="tpps", bufs=1,
                                      space="PSUM") as tpps, \
                         tc.tile_pool(name="esb", bufs=3) as esb, \
                         tc.tile_pool(name="episb", bufs=1) as episb:
                        for w in range(NWIN):
                            base_t = w * TW
                            ecol0 = w * TWE
                            ybufW = gsb.tile([128, TW, YW], BF16, tag="ybufW")
                            x0bW = gsb.tile([128, TWE], BF16, tag="x0bW")
                            goff = w * WCOL
                            for c0, cw in GCH:
                                gsl = slice(goff, goff + cw // 16)
                                nc.gpsimd.dma_gather(
                                    ybufW[:, c0 // 128:c0 // 128 + cw // 128,
                                          :],
                                    agout.ap()[:, 0:YW],
                                    idxs[:, gsl],
                                    cw, cw, YW, elem_step=TROW)
                                nc.gpsimd.dma_gather(
                                    bass.AP(tensor=x0bW[:].tensor,
                                            offset=x0bW[:].offset + c0,
                                            ap=[x0bW[:].ap[0], [cw, 1],
                                                [1, cw]]),
                                    agout.ap()[:, YW:TROW],
                                    idxs[:, gsl],
                                    cw, cw, C, elem_step=TROW,
                                    transpose=True)
                                goff += cw // 16
                            ohenW = wsb.tile([128, TWE], BF16, tag="ohenW")
                            ohneW = wsb.tile([128, TWE], BF16, tag="ohneW")
                            for ti in range(TW):
                                t = base_t + ti
                                tsl = slice(ti * 128, (ti + 1) * 128)
                                nc.gpsimd.tensor_scalar(
                                    out=ohenW[:, tsl], in0=iotart[:],
                                    scalar1=dlncs[:, t:t + 1], scalar2=None,
                                    op0=ISEQ)
                                tpo = tpps.tile([128, 128], BF16, tag="tp")
                                nc.tensor.transpose(out=tpo[:],
                                                    in_=ohenW[:, tsl],
                                                    identity=identt[:])
                                nc.scalar.activation(out=ohneW[:, tsl],
                                                     in_=tpo[:],
                                                     func=AF.Identity)
                            # pre-activation logits feats, [C', edge] chunks
                            preW = wsb.tile([C, TWE], BF16, tag="preW")
                            lg = lgps.tile([128, 512], F32, tag="lg")
                            for c0, cw in GCH:
                                pre = pps.tile([C, 512], F32, tag="pre")
                                nc.tensor.matmul(
                                    pre[:, 0:cw], lhsT=wa1t[:, l, 2, :],
                                    rhs=e_T[:, ecol0 + c0:ecol0 + c0 + cw],
                                    start=True, stop=False)
                                nc.tensor.matmul(
                                    pre[:, 0:cw], lhsT=wa1t[:, l, 0, :],
                                    rhs=x0bW[:, c0:c0 + cw],
                                    start=False, stop=False)
                                nsub = cw // 128
                                for s in range(nsub):
                                    ti = c0 // 128 + s
                                    nc.tensor.matmul(
                                        pre[:, s * 128:(s + 1) * 128],
                                        lhsT=t1w[:, w, :],
                                        rhs=ohneW[:, ti * 128:
                                                  (ti + 1) * 128],
                                        start=False, stop=(s == nsub - 1))
                                nc.scalar.activation(
                                    out=preW[:, c0:c0 + cw],
                                    in_=pre[:, 0:cw], func=AF.Identity)
                            # gate logits per tile into lg[:, 0:TW8]
                            for ti in range(TW):
                                t = base_t + ti
                                nc.tensor.matmul(
                                    lg[:, ti * 8:(ti + 1) * 8],
                                    lhsT=e_T[:, t * 128:(t + 1) * 128],
                                    rhs=wgt[:, l, :], start=True, stop=True)
                            # batched sigmoids (one table load): silu(pre)
                            # = pre * sigmoid(pre), chunked mult on DVE
                            gtw = episb.tile([128, TW8], BF16, tag="gtw")
                            for c0 in range(0, TWE, 512):
                                cw = min(512, TWE - c0)
                                sgc = esb.tile([C, 512], BF16, tag="sgc")
                                nc.scalar.activation(
                                    out=sgc[:, 0:cw], in_=preW[:, c0:c0 + cw],
                                    func=AF.Sigmoid)
                                nc.vector.tensor_tensor(
                                    out=preW[:, c0:c0 + cw],
                                    in0=preW[:, c0:c0 + cw],
                                    in1=sgc[:, 0:cw], op=MUL)
                            nc.scalar.activation(out=gtw[:], in_=lg[:, 0:TW8],
                                                 func=AF.Sigmoid)
                            # attn logits per tile into lg[:, 256:256+TW8]
                            for ti in range(TW):
                                nc.tensor.matmul(
                                    lg[:, 256 + ti * 8:256 + (ti + 1) * 8],
                                    lhsT=preW[:, ti * 128:(ti + 1) * 128],
                                    rhs=wa2t[:, l, :], start=True, stop=True)
                            exw = episb.tile([128, TW8], BF16, tag="exw")
                            nc.scalar.activation(out=exw[:],
                                                 in_=lg[:, 256:256 + TW8],
                                                 func=AF.Exp)
                            wbw = episb.tile([128, TW8], F32, tag="wbw")
                            nc.vector.tensor_tensor(out=wbw[:], in0=exw[:],
                                                    in1=gtw[:], op=MUL)
                            # scatter: agg[n, (k,h,v)] + sden[n, h]
                            aggs = agps.tile([128, YW], F32, tag="aggs")
                            sden = agps.tile([128, 8], F32, tag="sden")
                            for ti in range(TW):
                                tsl = slice(ti * 128, (ti + 1) * 128)
                                msk = esb.tile([128, YW], BF16, tag="msk")
                                nc.vector.tensor_tensor(
                                    out=msk[:].rearrange(
                                        "e (k h v) -> e k h v", k=K, h=H),
                                    in0=ybufW[:, ti, :].rearrange(
                                        "e (k h v) -> e k h v", k=K, h=H),
                                    in1=_bc4(wbw[:], ti * 8), op=MUL)
                                st = (ti == 0)
                                sp = (ti == TW - 1)
                                nc.tensor.matmul(aggs[:, 0:512],
                                                 lhsT=ohenW[:, tsl],
                                                 rhs=msk[:, 0:512],
                                                 start=st, stop=sp)
                                nc.tensor.matmul(aggs[:, 512:1024],
                                                 lhsT=ohenW[:, tsl],
                                                 rhs=msk[:, 512:1024],
                                                 start=st, stop=sp)
                                nc.tensor.matmul(aggs[:, 1024:YW],
                                                 lhsT=ohenW[:, tsl],
                                                 rhs=msk[:, 1024:YW],
                                                 start=st, stop=sp)
                                nc.tensor.matmul(sden[:],
                                                 lhsT=ohenW[:, tsl],
                                                 rhs=exw[:, ti * 8:
                                                         (ti + 1) * 8],
                                                 start=st, stop=sp)
                            # ----- window epilogue -----
                            rcp = episb.tile([128, H], F32, tag="rcp")
                            nc.vector.tensor_scalar_add(
                                out=rcp[:], in0=sden[:],
                                scalar1=1e-9)
                            nc.vector.reciprocal(out=rcp[:], in_=rcp[:])
                            aggn = episb.tile([128, YW], BF16, tag="aggn")
                            nc.vector.tensor_tensor(
                                out=aggn[:].rearrange(
                                    "n (k h v) -> n k h v", k=K, h=H),
                                in0=aggs[:, 0:YW].rearrange(
                                    "n (k h v) -> n k h v", k=K, h=H),
                                in1=_bc4(rcp[:], 0), op=MUL)
                            for k in range(K):
                                tpe = tpps.tile([128, 128], BF16, tag="tp")
                                nc.tensor.transpose(
                                    out=tpe[:],
                                    in_=aggn[:, k * 128:(k + 1) * 128],
                                    identity=identt[:])
                                aT = esb.tile([128, 128], BF16, tag="aT")
                                nc.scalar.activation(out=aT[:], in_=tpe[:],
                                                     func=AF.Identity)
                                dk = tpps.tile([C, 128], F32, tag="dk")
                                nc.tensor.matmul(dk[:], lhsT=wot[:, l, :],
                                                 rhs=aT[:],
                                                 start=True, stop=True)
                                xsl = slice(k * NLOCP + w * 128,
                                            k * NLOCP + (w + 1) * 128)
                                nc.vector.tensor_tensor(
                                    out=x_T[:, xsl], in0=x_T[:, xsl],
                                    in1=dk[:], op=ADD)

                    # ---------- FFN ----------
                    rms_norm(xn_bf)
                    with tc.tile_pool(name="fps", bufs=2,
                                      space="PSUM") as fps, \
                         tc.tile_pool(name="dps", bufs=2,
                                      space="PSUM") as dps, \
                         tc.tile_pool(name="fsb", bufs=3) as fsb:
                        for j in range(18):
                            c0 = j * 384
                            nsl = slice((j % 2) * 384, (j % 2) * 384 + 384)
                            dlt = dps.tile([C, 384], F32, tag="dlt")
                            for fc in range(4):
                                hp = fps.tile([128, 384], F32, tag="hp")
                                nc.tensor.matmul(
                                    hp[:],
                                    lhsT=wf1t[:, l, fc * 128:(fc + 1) * 128],
                                    rhs=xn_bf[:, c0:c0 + 384],
                                    start=True, stop=True)
                                if j < 2:
                                    nc.scalar.activation(
                                        out=sig0[:, fc, nsl],
                                        in_=hp[:], func=AF.Sigmoid)
                                hb = fsb.tile([128, 384], BF16, tag="hb")
                                nc.vector.tensor_tensor(
                                    out=hb[:], in0=hp[:],
                                    in1=sig0[:, fc, nsl], op=MUL)
                                nc.tensor.matmul(
                                    dlt[:], lhsT=wf2t[:, l, fc, :], rhs=hb[:],
                                    start=(fc == 0), stop=(fc == 3))
                            nc.vector.tensor_tensor(
                                out=x_T[:, c0:c0 + 384],
                                in0=x_T[:, c0:c0 + 384],
                                in1=dlt[:], op=ADD)

                # ---------- final norm + output ----------
                rms_norm(x_T)
                nc.sync.dma_start(out=out_d.ap(), in_=x_T[:])

    nc.compile()
    return nc


# ============================================================
# host preprocessing + runner
# ============================================================

_CACHE = {}


def _prep(inputs):
    """Index-only host preprocessing; returns (TW, in_maps, gslot)."""
    atom_feats = np.asarray(inputs["atom_feats"]).astype(np.int64)
    bond_feats = np.asarray(inputs["bond_feats"]).astype(np.int64)
    edge_index = np.asarray(inputs["edge_index"]).astype(np.int64)
    edge_distance = np.asarray(inputs["edge_distance"]).astype(np.float32)

    src, dst = edge_index[0], edge_index[1]
    NW = NCORES * NWIN            # 48 windows global

    # ---- degree-balanced node -> (window, slot) assignment ----
    deg = np.bincount(dst, minlength=N)
    order_nodes = np.argsort(-deg, kind="stable")
    win = np.empty(N, np.int64)
    pos = np.empty(N, np.int64)
    r = np.arange(N)
    win[order_nodes] = r % NW
    pos[order_nodes] = r // NW     # 0..124 (125 nodes per window)
    gslot = win * 128 + pos        # global slot id, 0..6143

    wl = np.bincount(win[dst], minlength=NW)
    TW = _cdiv(int(wl.max()), 128)
    TW += TW % 2                   # even (gather half-windows)
    NT = NWIN * TW
    ECAP = NT * 128
    TWE = TW * 128

    # ---- edge -> slot assignment (sorted by (window, src)) ----
    wid_e = win[dst]
    order_e = np.lexsort((gslot[src], wid_e))
    we_s = wid_e[order_e]
    starts = np.zeros(NW + 1, np.int64)
    np.cumsum(np.bincount(we_s, minlength=NW), out=starts[1:])
    rank = np.arange(E) - starts[we_s]
    eslot_g = we_s * TWE + rank            # global edge-slot (48*TWE space)

    srcslot = np.zeros(NW * TWE, np.int64)
    dlnf = np.full(NW * TWE, -1.0, np.float32)
    distf = np.zeros(NW * TWE, np.float32)
    bff = np.full((3, NW * TWE), -1.0, np.float32)
    srcslot[eslot_g] = gslot[src[order_e]]
    dlnf[eslot_g] = (gslot[dst[order_e]] % 128).astype(np.float32)
    distf[eslot_g] = edge_distance[order_e]
    bff[:, eslot_g] = bond_feats[order_e].T.astype(np.float32)

    # ---- weights ----
    f32 = np.float32
    bf16 = ml_dtypes.bfloat16
    W_rbf1 = np.asarray(inputs["W_rbf1"], f32)
    W_rbf2 = np.asarray(inputs["W_rbf2"], f32)
    b_rbf1 = np.asarray(inputs["b_rbf1"], f32)
    b_rbf2 = np.asarray(inputs["b_rbf2"], f32)
    Wa1 = np.asarray(inputs["Wa1"], f32)
    Wa2 = np.asarray(inputs["Wa2"], f32)
    Wv = np.asarray(inputs["Wv"], f32)
    Wg = np.asarray(inputs["Wg"], f32)
    Wo = np.asarray(inputs["Wo"], f32)
    Wf1 = np.asarray(inputs["Wf1"], f32)
    Wf2 = np.asarray(inputs["Wf2"], f32)
    atom_emb = np.asarray(inputs["atom_emb"], f32)
    bond_emb = np.asarray(inputs["bond_emb"], f32)

    centers = np.linspace(0.0, RMAX, B).astype(f32)
    cneg = (-centers / GW).reshape(4, 128).T.copy()       # [128, 4]

    common = {
        "wrbf1": W_rbf1.astype(bf16),
        "wrbf2": W_rbf2.astype(bf16),
        "brbf1": b_rbf1.reshape(C, 1),
        "brbf2": b_rbf2.reshape(C, 1),
        "cneg": np.ascontiguousarray(cneg),
        "atomtab": atom_emb.reshape(40, C).copy(),
        "bondtab": bond_emb.reshape(24, C).astype(bf16),
        "wa1": np.ascontiguousarray(Wa1.reshape(L, 3, C, C)).astype(bf16),
        "wa2": Wa2.astype(bf16),
        "wg": Wg.astype(bf16),
        "wv": Wv.astype(bf16),
        "wo": Wo.astype(bf16),
        "wf1": Wf1.astype(bf16),
        "wf2": Wf2.astype(bf16),
        "iotac": np.tile(np.arange(128, dtype=f32), (128, 1)),
        "iota8": (np.arange(24) % 8).astype(f32).reshape(24, 1),
    }

    # atom one-hot per core (cols = local slots, pads stay zero)
    core_n = gslot // NLOCP
    loc_n = gslot % NLOCP
    aoh_all = np.zeros((NCORES, 40, NLOCP), f32)
    for f in range(4):
        aoh_all[core_n, f * 10 + atom_feats[:, f], loc_n] = 1.0

    # wrapped int16 gather indices: per gather chunk (<=512 idxs), idxs
    # wrap into 16 rows x cw/16 cols, chunks packed consecutively,
    # replicated to 128 partitions.
    gch = _gchunks(TW)
    in_maps = []
    for d in range(NCORES):
        sl = slice(d * ECAP, (d + 1) * ECAP)
        ss = srcslot[sl].astype(np.int16)
        idx = np.zeros((16, ECAP // 16), np.int16)
        col = 0
        for w in range(NWIN):
            for c0, cw in gch:
                q = np.arange(cw)
                chunk = ss[w * TWE + c0: w * TWE + c0 + cw]
                idx[q % 16, col + q // 16] = chunk
                col += cw // 16
        idx = np.tile(idx, (8, 1))

        m = dict(common)
        m.update({
            "aoh": aoh_all[d],
            "dist": distf[sl].reshape(1, ECAP),
            "bfr": np.ascontiguousarray(bff[:, sl]),
            "dlnc": np.ascontiguousarray(
                dlnf[sl].reshape(NT, 128).T),
            "idx": idx,
        })
        in_maps.append(m)
    return TW, in_maps, gslot


def _get_nc(TW, reps=1):
    key = ('nc', TW, reps)
    if key not in _CACHE:
        _CACHE[key] = build_program(TW, reps=reps)
    return _CACHE[key]


def _make_runner(nc, reps):
    """jit-compiled SPMD runner that chains the NEFF `reps` times
    back-to-back, so (T(reps) - T(1)) / (reps - 1) cancels host/axon
    dispatch overhead."""
    import jax
    from jax.sharding import Mesh, PartitionSpec
    from jax.experimental.shard_map import shard_map
    from concourse import bass2jax
    import concourse.mybir as mb

    bass2jax.install_neuronx_cc_hook()
    part_name = (nc.partition_id_tensor.name
                 if nc.partition_id_tensor else None)
    in_names, out_names, out_avals, zero_outs = [], [], [], []
    for alloc in nc.m.functions[0].allocations:
        if not isinstance(alloc, mybir.MemoryLocationSet):
            continue
        name = alloc.memorylocations[0].name
        if alloc.kind == "ExternalInput":
            if name != part_name:
                in_names.append(name)
        elif alloc.kind == "ExternalOutput":
            out_names.append(name)
            shape = tuple(alloc.tensor_shape)
            dtype = mb.dt.np(alloc.dtype)
            out_avals.append(jax.core.ShapedArray(shape, dtype))
            zero_outs.append(np.zeros(shape, dtype))
    n_params = len(in_names)
    all_names = list(in_names) + list(out_names)
    if part_name is not None:
        all_names.append(part_name)

    def _body(*args):
        o = list(args[n_params:])
        for _ in range(reps):
            ops = list(args[:n_params]) + o
            if part_name is not None:
                ops.append(bass2jax.partition_id_tensor())
            o = list(bass2jax._bass_exec_p.bind(
                *ops,
                out_avals=tuple(out_avals),
                in_names=tuple(all_names),
                out_names=tuple(out_names),
                lowering_input_output_aliases=(),
                sim_require_finite=True,
                sim_require_nnan=True,
                nc=nc))
        return tuple(o)

    devices = jax.devices()[:NCORES]
    mesh = Mesh(np.asarray(devices), ("core",))
    n_outs = len(out_names)
    in_specs = (PartitionSpec("core"),) * (n_params + n_outs)
    out_specs = (PartitionSpec("core"),) * n_outs
    donate = tuple(range(n_params, n_params + n_outs))
    fn = jax.jit(
        shard_map(_body, mesh=mesh, in_specs=in_specs,
                  out_specs=out_specs, check_rep=False),
        donate_argnums=donate, keep_unused=True)
    return fn, mesh, in_names, out_names, out_avals, zero_outs


def _concat_inputs(in_maps, in_names):
    return [np.concatenate([np.asarray(in_maps[c][n]) for c in range(NCORES)],
                           axis=0) for n in in_names]


def _unpack_out(arrs, gslot):
    # arrs[0]: [NCORES*C, KNP] -> full [N, K, C]
    xo = np.asarray(arrs[0]).reshape(NCORES, C, K, NLOCP)
    flat = np.ascontiguousarray(xo.transpose(0, 3, 2, 1)).reshape(
        NSLOT, K, C)
    return flat[gslot].astype(np.float32)


def _run(TW, in_maps, gslot, reps_timing=0):
    """Returns (out, timing_info)."""
    import jax, time
    nc = _get_nc(TW)
    key = (TW, 1)
    if key not in _CACHE:
        _CACHE[key] = _make_runner(nc, 1)
    fn1, mesh, in_names, out_names, out_avals, zero_outs = _CACHE[key]
    cin = _concat_inputs(in_maps, in_names)
    czo = [np.zeros((NCORES * z.shape[0], *z.shape[1:]), z.dtype)
           for z in zero_outs]
    outs = fn1(*cin, *czo)
    jax.block_until_ready(outs)
    result = _unpack_out(outs, gslot)

    timing = None
    if reps_timing:
        from jax.sharding import NamedSharding, PartitionSpec
        shard = NamedSharding(mesh, PartitionSpec("core"))
        cin_dev = [jax.device_put(a, shard) for a in cin]

        def seq_times(f, n):
            ts = []
            for _ in range(n):
                z = [jax.device_put(np.zeros_like(a), shard) for a in czo]
                jax.block_until_ready(z)
                t0 = time.perf_counter()
                o = f(*cin_dev, *z)
                jax.block_until_ready(o)
                ts.append(time.perf_counter() - t0)
            return ts

        seq_times(fn1, 2)  # warm
        ts1 = seq_times(fn1, 10)
        R = reps_timing
        tsR = None
        if R > 1:
            ncR = _get_nc(TW, reps=R)
            kr = ("fn", TW, R)
            if kr not in _CACHE:
                _CACHE[kr] = _make_runner(ncR, 1)
            fnR = _CACHE[kr][0]
            seq_times(fnR, 1)  # warm/compile
            tsR = seq_times(fnR, 10)
            # medians: robust to dispatch-time outliers in either sample
            med = lambda v: sorted(v)[len(v) // 2]
            per_iter = max(med(tsR) - med(ts1), 0.0) / (R - 1)
        else:
            per_iter = min(ts1)
        timing = dict(ts1=ts1, tsR=tsR, reps=R, per_iter=per_iter)
    return result, timing


def kernel(**inputs):
    TW, in_maps, gslot = _prep(inputs)
    out, _ = _run(TW, in_maps, gslot)
    return out
